# revision 19
# baseline (speedup 1.0000x reference)
"""MultiHeadAttention forward on 8 Trainium2 NeuronCores.

Tensor-parallel over heads: each core owns 2 of 16 heads (d_loc=256 of the
2048 QKV output columns, and the matching 256 rows of Wo). Each core
computes a full-shape partial output; the host sums the 8 partials and
adds bo (+ bv @ Wo for the folded V bias).

Problem shape: x [2, 2048, 2048], 16 heads, d_k = 128; device math in
bf16 (tolerance 2e-2; bf16 lands ~5e-3) with fp32 PSUM accumulation.

PE cost model (measured on this part): bf16 matmuls stream 0.5
cycles/row (2x the fp32r rate) and every matmul pays a fixed ~120 ns
serial LDWEIGHTS. The kernel therefore (a) keeps every matmul at the
512-wide PSUM-bank limit so the count is minimal, (b) emits matmuls
sharing a stationary operand back-to-back and re-enables walrus's
redundant-load-weight elision (--enable-ldw-opt, default-off in this
toolchain) so the second of each pair skips its weight load, and (c)
projects V transposed (512-wide streams like Q/K) then DMA-XBAR
transposes it into packed [128,128] natural tiles (issued from the
Scalar/Vector queues, which are otherwise idle at that point).

Softmax: scores transposed ST[tk, tq]; exp on ScalarE in [128,1024]
tiles; denominator = DVE bf16 accumulation of exp tiles + one
ones-matmul partition reduction per (head, chunk) written into the
score PSUM pool; reciprocal on DVE.

Emission order keeps the in-order PE fed while ScalarE paces attention
(closure-granular fills ride in each k-tile's exp->AV latency window):
proj(b0) | attn(b0)+proj(b1) fills | drain + outproj(b0) |
attn(b1)+outproj fills | tail.
"""

import functools
from collections import deque
from contextlib import ExitStack

import numpy as np

D_MODEL = 2048
NUM_HEADS = 16
DK = 128
B = 2
T = 2048
BT = B * T
N_CORES = 8
H_LOC = NUM_HEADS // N_CORES  # 2 heads per core
D_LOC = H_LOC * DK  # 256
C_TILES = D_MODEL // 128  # 16
TQ = 512  # tq chunk width (one PSUM bank in fp32)
NCH = T // TQ  # 4 chunks per batch
TK_TILES = T // 128  # 16
NDEST = 6  # v0 v1 k0 k1 q0 q1 projection destinations


def _ldw_sig(inst):
    import json as _json

    w = inst["ins"][0]
    return _json.dumps(
        [
            w.get("memref"),
            w.get("offset"),
            w.get("ap"),
            w.get("dtype"),
            inst.get("tile_position"),
            inst.get("tile_size"),
            inst.get("perf_mode"),
            inst.get("is_transpose"),
        ],
        sort_keys=True,
    )


def _dedup_ldweights(bj):
    """Drop a Ldweights whose weights AP matches the previous load on the
    PE stream, when it carries no waits/updates: the stationary registers
    still hold those exact weights, so the paired (non-self-loading)
    Matmult runs unchanged and the PE saves the ~120 ns serial reload.
    Every matmul here pays that load, so kernel loops are arranged to
    produce such adjacent same-stationary pairs."""
    removed = 0
    for fn in bj.get("functions", []):
        for blk in fn.get("blocks", []):
            out = []
            last = None
            for inst in blk["instructions"]:
                if inst.get("engine") == "PE":
                    op = inst.get("opcode")
                    if op == "Ldweights":
                        si = inst.get("sync_info") or {}
                        sig = _ldw_sig(inst)
                        if (
                            sig == last
                            and not si.get("on_wait")
                            and not si.get("on_update")
                        ):
                            removed += 1
                            continue
                        last = sig
                    elif op == "Matmult" and not inst.get("ldweights", False):
                        pass
                    elif op == "EventSemaphore":
                        pass
                    else:
                        last = None
                out.append(inst)
            blk["instructions"] = out
    return removed


def _patch_ldw_dedup():
    """Apply _dedup_ldweights to the BIR json on disk just before walrus."""
    import json
    import os

    import concourse.bass_utils as BU

    if getattr(BU, "_ldw_dedup_patched", False):
        return
    orig = BU.bir_verify_and_optimise

    def wrapped(tmpdir, inp="bir.json", *args, **kwargs):
        try:
            p = os.path.join(tmpdir, inp)
            with open(p) as f:
                bj = json.load(f)
            if _dedup_ldweights(bj):
                with open(p, "w") as f:
                    json.dump(bj, f)
        except Exception:
            pass
        return orig(tmpdir, inp, *args, **kwargs)

    BU.bir_verify_and_optimise = wrapped
    BU._ldw_dedup_patched = True


def _body(ctx, tc, xT, wqkv, bqk, wo, y):
    import concourse.bass as bass  # noqa: F401
    from concourse import mybir

    nc = tc.nc
    f32 = mybir.dt.float32
    bf16 = mybir.dt.bfloat16
    Exp = mybir.ActivationFunctionType.Exp
    inv_sqrt_dk = 1.0 / float(np.sqrt(DK))

    # ---------------- resident tensors ----------------
    wpool = ctx.enter_context(tc.tile_pool(name="wpool", bufs=1))
    x_pool = ctx.enter_context(tc.tile_pool(name="x_pool", bufs=48))

    w_tiles = []
    xt_pre = []
    for i in range(C_TILES):
        xti = x_pool.tile([128, TQ], bf16, tag="xt", name=f"xtpre{i}")
        nc.sync.dma_start(out=xti, in_=xT[i * 128 : (i + 1) * 128, 0:TQ])
        xt_pre.append(xti)
        wt = wpool.tile([128, 3 * D_LOC], bf16, tag=f"w{i}", name=f"w{i}")
        nc.sync.dma_start(out=wt, in_=wqkv[i * 128 : (i + 1) * 128, :])
        w_tiles.append(wt)
    bqk_sb = wpool.tile([128, 4], f32, tag="bqk", name="bqk")
    nc.sync.dma_start(out=bqk_sb, in_=bqk[:, :])

    wo_tiles = []
    for d in range(2):
        wot = wpool.tile([128, D_MODEL], bf16, tag=f"wo{d}", name=f"wo{d}")
        nc.sync.dma_start(out=wot, in_=wo[d * 128 : (d + 1) * 128, :])
        wo_tiles.append(wot)

    ones = wpool.tile([128, 128], bf16, tag="ones", name="ones")
    nc.vector.memset(ones, 1.0)

    # ---------------- pools ----------------
    qkv_pool = ctx.enter_context(tc.tile_pool(name="qkv_pool", bufs=1))
    av_pool = ctx.enter_context(tc.tile_pool(name="av_pool", bufs=1))
    es_pool = ctx.enter_context(tc.tile_pool(name="es_pool", bufs=6))
    acc_pool = ctx.enter_context(tc.tile_pool(name="acc_pool", bufs=4))
    rc_pool = ctx.enter_context(tc.tile_pool(name="rc_pool", bufs=4))
    y_pool = ctx.enter_context(tc.tile_pool(name="y_pool", bufs=4))

    # PSUM (8 banks): ps_po 2x[128,512] (proj pairs / outproj) +
    # ps_s 2x[128,1024] (scores + denominators) + ps_av 1x[128,1024].
    ps_po = ctx.enter_context(tc.tile_pool(name="ps_po", bufs=2, space="PSUM"))
    ps_s = ctx.enter_context(tc.tile_pool(name="ps_s", bufs=2, space="PSUM"))
    ps_av = ctx.enter_context(tc.tile_pool(name="ps_av", bufs=1, space="PSUM"))

    qT, kT, v_pk, avT = {}, {}, {}, {}

    def alloc_batch(b):
        qT[b] = [
            qkv_pool.tile([128, T], bf16, tag=f"qT{d}", name=f"qT{d}_{b}", bufs=2)
            for d in range(2)
        ]
        kT[b] = [
            qkv_pool.tile([128, T], bf16, tag=f"kT{d}", name=f"kT{d}_{b}", bufs=2)
            for d in range(2)
        ]
        # vT: V projected transposed like Q/K, then DMA-transposed into
        # packed natural tiles v_pk[t][h].
        vT = [
            qkv_pool.tile([128, T], bf16, tag=f"vT{d}", name=f"vT{d}_{b}", bufs=2)
            for d in range(2)
        ]
        v_pk[b] = [
            [
                qkv_pool.tile(
                    [128, 128], bf16, tag=f"v{t}_{h}", name=f"v{t}_{h}_{b}", bufs=2
                )
                for h in range(H_LOC)
            ]
            for t in range(TK_TILES)
        ]
        avT[b] = [
            av_pool.tile([128, T], bf16, tag=f"avT{d}", name=f"avT{d}_{b}", bufs=2)
            for d in range(2)
        ]
        return vT

    vT_b = {}
    xt_chunks = {}
    vt_deferred = []

    def emit_xt_dma(b, ch):
        t0 = b * T + ch * TQ
        xt = []
        for i in range(C_TILES):
            xti = x_pool.tile([128, TQ], bf16, tag="xt", name=f"xt{b}_{ch}_{i}")
            nc.sync.dma_start(
                out=xti, in_=xT[i * 128 : (i + 1) * 128, t0 : t0 + TQ]
            )
            xt.append(xti)
        return xt

    def finish_proj(b, ch, j, ps):
        # j -> (v0, v1, k0, k1, q0, q1); v first so the transposes clear
        # early, k before q so attention deps clear early.
        # wqkv column order is q0 q1 k0 k1 v0 v1.
        dest = (vT_b[b][0], vT_b[b][1], kT[b][0], kT[b][1], qT[b][0], qT[b][1])[j]
        wcol = (4, 5, 2, 3, 0, 1)[j]
        sl = dest[:, ch * TQ : (ch + 1) * TQ]
        if j < 2:
            nc.vector.tensor_copy(sl, ps)
            # XBAR-transpose the finished 128-col blocks into packed natural
            # tiles. The descriptor-gen costs ~1.2us per transpose on the
            # issuing queue, so b0's go to ScalarE (idle during proj b0) and
            # b1's are deferred to S3 (split across Sync/ScalarE, both idle
            # there) — never blocking regular DMAs or S2's exp pacing.
            h = j
            for ts in range(TQ // 128):
                t_idx = ch * (TQ // 128) + ts
                if b == 0:
                    nc.scalar.dma_start(
                        out=v_pk[b][t_idx][h],
                        in_=dest[:, t_idx * 128 : (t_idx + 1) * 128],
                        transpose=True,
                    )
                else:
                    vt_deferred.append((b, t_idx, h, dest))
        else:
            # PSUM -> SBUF with per-partition bias add (q/k only)
            nc.vector.tensor_scalar_add(sl, ps, bqk_sb[:, wcol : wcol + 1])

    def proj_closures(b, chp, j):
        # One closure per contraction step, covering the chunk pair
        # (2*chp, 2*chp+1) with the SAME stationary w-tile back-to-back so
        # ldw-opt elides the second load.
        st = {}
        wcol = (4, 5, 2, 3, 0, 1)[j]

        def mk(i):
            def go():
                if i == 0:
                    for c in range(2):
                        st[c] = ps_po.tile(
                            [128, TQ], f32, tag="po", name=f"psp{b}_{chp}_{j}_{c}"
                        )
                for c in range(2):
                    nc.tensor.matmul(
                        st[c],
                        w_tiles[i][:, wcol * 128 : (wcol + 1) * 128],
                        xt_chunks[(b, 2 * chp + c)][i],
                        start=(i == 0),
                        stop=(i == C_TILES - 1),
                    )
                if i == C_TILES - 1:
                    for c in range(2):
                        finish_proj(b, 2 * chp + c, j, st[c])

            return go

        return [("pe", mk(i)) for i in range(C_TILES)]

    def o_t_closures(b, t):
        # Output projection for one 128-row tile of y as two self-contained
        # closures (one per 1024-col half): each runs both d-stationaries
        # over a pair of PSUM banks (same-stationary matmuls adjacent).
        row0 = b * T + t * 128

        def mk(half):
            def go():
                ys = y_pool.tile(
                    [128, D_MODEL // 2],
                    bf16,
                    tag="ystage",
                    name=f"ys{b}_{t}_{half}",
                )
                ps = [
                    ps_po.tile(
                        [128, TQ], f32, tag="po", name=f"pso{b}_{t}_{half}_{q}"
                    )
                    for q in range(2)
                ]
                for d in range(2):
                    for q in range(2):
                        nch_i = half * 2 + q
                        nc.tensor.matmul(
                            ps[q],
                            avT[b][d][:, t * 128 : (t + 1) * 128],
                            wo_tiles[d][:, nch_i * TQ : (nch_i + 1) * TQ],
                            start=(d == 0),
                            stop=(d == 1),
                        )
                for q in range(2):
                    nc.vector.tensor_copy(ys[:, q * TQ : (q + 1) * TQ], ps[q])
                nc.sync.dma_start(
                    out=y[
                        row0 : row0 + 128,
                        half * (D_MODEL // 2) : (half + 1) * (D_MODEL // 2),
                    ],
                    in_=ys,
                )

            return go

        return [("pe", mk(half)) for half in range(2)]

    def emit_attn_unit(b, h, chp, fill_q, per_tk=2):
        # One unit covers the 1024-query chunk pair (2*chp, 2*chp+1).
        # fill_q holds closures each emitting ~1-2 independent PE matmuls;
        # a few are popped per k-tile so the in-order PE has work while it
        # waits on ScalarE's exp for the pav matmuls.
        pav = ps_av.tile([128, 2 * TQ], f32, tag="av", name=f"pav{b}_{h}_{chp}")
        acc = [
            acc_pool.tile([128, TQ], bf16, tag="acc", name=f"acc{b}_{h}_{chp}_{c}")
            for c in range(2)
        ]
        q_sl = qT[b][h][:, chp * 2 * TQ : (chp + 1) * 2 * TQ]
        for tk in range(TK_TILES):
            pss = ps_s.tile(
                [128, 2 * TQ], f32, tag="s", name=f"pss{b}_{h}_{chp}_{tk}"
            )
            es = es_pool.tile(
                [128, 2 * TQ], bf16, tag="es", name=f"es{b}_{h}_{chp}_{tk}"
            )
            for c in range(2):
                nc.tensor.matmul(
                    pss[:, c * TQ : (c + 1) * TQ],
                    kT[b][h][:, tk * 128 : (tk + 1) * 128],
                    q_sl[:, c * TQ : (c + 1) * TQ],
                    start=True,
                    stop=True,
                )
            nc.scalar.activation(es, pss, Exp, scale=inv_sqrt_dk)
            # fills ride in the exp->pav latency window
            done = 0
            while done < per_tk and fill_q:
                kind, c_ = fill_q.popleft()
                c_()
                if kind == "pe":
                    done += 1
            for c in range(2):
                nc.tensor.matmul(
                    pav[:, c * TQ : (c + 1) * TQ],
                    v_pk[b][tk][h],
                    es[:, c * TQ : (c + 1) * TQ],
                    start=(tk == 0),
                    stop=(tk == TK_TILES - 1),
                )
            with nc.allow_low_precision("softmax denominator partials, bf16"):
                if tk == 0:
                    nc.vector.tensor_copy(acc[0], es[:, :TQ])
                    nc.vector.tensor_copy(acc[1], es[:, TQ:])
                else:
                    nc.vector.tensor_add(acc[0], acc[0], es[:, :TQ])
                    nc.vector.tensor_add(acc[1], acc[1], es[:, TQ:])
        # a few extra fills cover the acc-chain drain before the denominators
        done = 0
        while done < 4 and fill_q:
            kind, c_ = fill_q.popleft()
            c_()
            if kind == "pe":
                done += 1
        # denominators: both chunks' partition reductions share the ones
        # stationary and a score-pool tile (halves are separate banks)
        pdn = ps_s.tile([128, 2 * TQ], f32, tag="s", name=f"pdn{b}_{h}_{chp}")
        for c in range(2):
            nc.tensor.matmul(
                pdn[:, c * TQ : (c + 1) * TQ],
                ones[:, 0:128],
                acc[c],
                start=True,
                stop=True,
            )
        for c in range(2):
            ch = 2 * chp + c
            rc = rc_pool.tile([128, TQ], f32, tag="rc", name=f"rc{b}_{h}_{ch}")
            nc.vector.reciprocal_approx_fast(
                out=rc, in_=pdn[:, c * TQ : (c + 1) * TQ]
            )
            nc.vector.tensor_mul(
                avT[b][h][:, ch * TQ : (ch + 1) * TQ],
                pav[:, c * TQ : (c + 1) * TQ],
                rc,
            )

    # ---------------- S1: projections for batch 0 ----------------
    vT_b[0] = alloc_batch(0)
    xt_chunks[(0, 0)] = xt_pre
    xt_chunks[(0, 1)] = emit_xt_dma(0, 1)
    for chp in range(NCH // 2):
        if chp == 0:
            xt_chunks[(0, 2)] = emit_xt_dma(0, 2)
            xt_chunks[(0, 3)] = emit_xt_dma(0, 3)
        for j in range(NDEST):
            for _, c in proj_closures(0, chp, j):
                c()

    # ---------------- S2: attn(b0) with proj(b1) fills ----------------
    vT_b[1] = alloc_batch(1)
    fq = deque()

    def dma_closure(b, ch):
        def go():
            xt_chunks[(b, ch)] = emit_xt_dma(b, ch)

        return ("free", go)

    fq.append(dma_closure(1, 0))
    fq.append(dma_closure(1, 1))
    for chp in range(NCH // 2):
        if chp == 0:
            fq.append(dma_closure(1, 2))
            fq.append(dma_closure(1, 3))
        for j in range(NDEST):
            fq.extend(proj_closures(1, chp, j))

    for chp in range(NCH // 2):
        for h in range(H_LOC):
            emit_attn_unit(0, h, chp, fq, per_tk=2)

    # ---------------- S3: drain proj(b1), then outproj(b0) halves ------
    while fq:
        fq.popleft()[1]()
    for n, (b_, t_, h_, dest_) in enumerate(vt_deferred):
        eng = nc.sync if n % 2 == 0 else nc.scalar
        eng.dma_start(
            out=v_pk[b_][t_][h_],
            in_=dest_[:, t_ * 128 : (t_ + 1) * 128],
            transpose=True,
        )
    vt_deferred.clear()
    for t in range(8):
        for _, c in o_t_closures(0, t):
            c()

    # ---------------- S4: attn(b1) with outproj fills ----------------
    oq = deque()
    for t in range(8, TK_TILES):
        oq.extend(o_t_closures(0, t))
    for chp in range(NCH // 2):
        for h in range(H_LOC):
            emit_attn_unit(1, h, chp, oq, per_tk=1)
            if chp == 0 and h == H_LOC - 1:
                for t in range(0, 8):
                    oq.extend(o_t_closures(1, t))
    for t in range(8, TK_TILES):
        oq.extend(o_t_closures(1, t))
    while oq:
        oq.popleft()[1]()


@functools.cache
def _build():
    from concourse import bacc
    import concourse.tile as tile
    from concourse import mybir

    _patch_ldw_dedup()
    nc = bacc.Bacc(
        "TRN2",
        target_bir_lowering=False,
        debug=False,
        enable_asserts=False,
        num_devices=N_CORES,
    )
    f32 = mybir.dt.float32
    bf16 = mybir.dt.bfloat16
    xT = nc.dram_tensor("xT", [D_MODEL, BT], bf16, kind="ExternalInput").ap()
    wqkv = nc.dram_tensor(
        "wqkv", [D_MODEL, 3 * D_LOC], bf16, kind="ExternalInput"
    ).ap()
    bqk = nc.dram_tensor("bqk", [128, 4], f32, kind="ExternalInput").ap()
    wo = nc.dram_tensor("wo", [D_LOC, D_MODEL], bf16, kind="ExternalInput").ap()
    y = nc.dram_tensor("y", [BT, D_MODEL], bf16, kind="ExternalOutput").ap()

    with tile.TileContext(nc) as tc:
        with ExitStack() as ctx:
            _body(ctx, tc, xT, wqkv, bqk, wo, y)
    nc.compile()
    return nc


def _shard_inputs(x, Wq, bq, Wk, bk, Wv, bv, Wo, bo):
    """Host-side sharding: returns per-core input maps."""
    import ml_dtypes

    bf = ml_dtypes.bfloat16
    f = np.float32
    xT = np.ascontiguousarray(
        np.asarray(x, f).reshape(BT, D_MODEL).T.astype(bf)
    )
    Wq, Wk, Wv, Wo = (np.asarray(a, f) for a in (Wq, Wk, Wv, Wo))
    bq, bk, bv = (np.asarray(a, f) for a in (bq, bk, bv))
    in_maps = []
    for c in range(N_CORES):
        sl = slice(c * D_LOC, (c + 1) * D_LOC)
        wqkv_pad = np.ascontiguousarray(
            np.concatenate([Wq[:, sl], Wk[:, sl], Wv[:, sl]], axis=1).astype(bf)
        )
        bqk_t = np.ascontiguousarray(
            np.stack(
                [
                    bq[sl][:128],
                    bq[sl][128:],
                    bk[sl][:128],
                    bk[sl][128:],
                ],
                axis=1,
            )
        )
        wo_loc = np.ascontiguousarray(Wo[sl, :].astype(bf))
        in_maps.append({"xT": xT, "wqkv": wqkv_pad, "bqk": bqk_t, "wo": wo_loc})
    return in_maps


def _run(in_maps, trace=False, **kwargs):
    from concourse.bass_utils import run_bass_kernel_spmd

    nc = _build()
    return run_bass_kernel_spmd(
        nc, in_maps, core_ids=list(range(N_CORES)), trace=trace, **kwargs
    )


def kernel(x, Wq, bq, Wk, bk, Wv, bv, Wo, bo):
    in_maps = _shard_inputs(x, Wq, bq, Wk, bk, Wv, bv, Wo, bo)
    res = _run(in_maps, trace=False)
    acc = np.zeros((BT, D_MODEL), np.float32)
    for rmap in res.results:
        acc += np.asarray(rmap["y"], dtype=np.float32)
    acc += np.asarray(bo, np.float32)[None, :]
    acc += (np.asarray(bv, np.float32) @ np.asarray(Wo, np.float32))[None, :]
    return acc.reshape(B, T, D_MODEL)


# revision 20
# speedup vs baseline: 1.1388x; 1.1388x over previous
"""MultiHeadAttention forward on 8 Trainium2 NeuronCores.

Tensor-parallel over heads: each core owns 2 of 16 heads (d_loc=256 of the
2048 QKV output columns, and the matching 256 rows of Wo). Each core
computes a full-shape partial output; the host sums the 8 partials and
adds bo (+ bv @ Wo for the folded V bias).

Problem shape: x [2, 2048, 2048], 16 heads, d_k = 128; device math in
bf16 (tolerance 2e-2; this kernel lands ~5e-3) with fp32 PSUM
accumulation. bf16 matmuls stream 0.5 cycles/row on this part (2x the
fp32r rate) and every matmul pays a fixed ~120 ns issue/load tax, so
all matmuls use the maximal 512-wide PSUM-bank output.

Layout: x fed pre-transposed (xT [C, B*T]); Q, K produced transposed
(QT/KT [d, t]); V natural [t, d]; scores transposed ST[tk, tq] =
matmul(lhsT=KT-tile, rhs=QT-chunk); no max-subtraction (|s| ~ 5).
exp on ScalarE in [128,1024] tiles (two k-tiles per activation).
Softmax denominator: DVE bf16 accumulation of the exp tiles + a single
ones-matmul partition reduction per (head, chunk); reciprocal on DVE;
1/denom applied to the accumulated AV^T chunks.

Emission interleaves coarse units so every engine stays fed (the bass
backend scheduler then fine-interleaves per-engine streams):
proj(b0) | attn(b0) + proj(b1) units | proj(b1) drain + outproj(b0) |
attn(b1) + outproj fills | tail.
"""

import functools
from contextlib import ExitStack

import numpy as np

D_MODEL = 2048
NUM_HEADS = 16
DK = 128
B = 2
T = 2048
BT = B * T
N_CORES = 8
H_LOC = NUM_HEADS // N_CORES  # 2 heads per core
D_LOC = H_LOC * DK  # 256
C_TILES = D_MODEL // 128  # 16
TQ = 512  # tq chunk width (one PSUM bank in fp32)
NCH = T // TQ  # 4 chunks per batch
TK_TILES = T // 128  # 16


def _body(ctx, tc, xT, wqkv, bqk, wo, y):
    import concourse.bass as bass  # noqa: F401
    from concourse import mybir

    nc = tc.nc
    f32 = mybir.dt.float32
    bf16 = mybir.dt.bfloat16
    Exp = mybir.ActivationFunctionType.Exp
    inv_sqrt_dk = 1.0 / float(np.sqrt(DK))

    # ---------------- resident tensors ----------------
    # Interleave the first x-chunk's tile loads with the weight loads so the
    # first projection matmuls can start after ~2 DMAs.
    wpool = ctx.enter_context(tc.tile_pool(name="wpool", bufs=1))
    x_pool = ctx.enter_context(tc.tile_pool(name="x_pool", bufs=20))

    w_tiles = []
    xt_pre = []
    for i in range(C_TILES):
        xti = x_pool.tile([128, TQ], bf16, tag="xt", name=f"xtpre{i}")
        nc.sync.dma_start(out=xti, in_=xT[i * 128 : (i + 1) * 128, 0:TQ])
        xt_pre.append(xti)
        wt = wpool.tile([128, 3 * D_LOC], bf16, tag=f"w{i}", name=f"w{i}")
        nc.sync.dma_start(out=wt, in_=wqkv[i * 128 : (i + 1) * 128, :])
        w_tiles.append(wt)
    bqk_sb = wpool.tile([128, 4], f32, tag="bqk", name="bqk")
    nc.sync.dma_start(out=bqk_sb, in_=bqk[:, :])

    wo_tiles = []
    for d in range(2):
        wot = wpool.tile([128, D_MODEL], bf16, tag=f"wo{d}", name=f"wo{d}")
        nc.sync.dma_start(out=wot, in_=wo[d * 128 : (d + 1) * 128, :])
        wo_tiles.append(wot)

    ones = wpool.tile([128, 128], bf16, tag="ones", name="ones")
    nc.vector.memset(ones, 1.0)

    # ---------------- pools ----------------
    qkv_pool = ctx.enter_context(tc.tile_pool(name="qkv_pool", bufs=1))
    av_pool = ctx.enter_context(tc.tile_pool(name="av_pool", bufs=1))
    es_pool = ctx.enter_context(tc.tile_pool(name="es_pool", bufs=6))
    acc_pool = ctx.enter_context(tc.tile_pool(name="acc_pool", bufs=2))
    rc_pool = ctx.enter_context(tc.tile_pool(name="rc_pool", bufs=2))
    y_pool = ctx.enter_context(tc.tile_pool(name="y_pool", bufs=3))

    # PSUM budget (8 banks): ps_po 2 (proj + outproj + denominator) +
    # ps_s 2x[128,1024] = 4 + ps_av 2.
    ps_po = ctx.enter_context(tc.tile_pool(name="ps_po", bufs=2, space="PSUM"))
    ps_s = ctx.enter_context(tc.tile_pool(name="ps_s", bufs=2, space="PSUM"))
    ps_av = ctx.enter_context(tc.tile_pool(name="ps_av", bufs=2, space="PSUM"))

    qT, kT, v_t, avT = {}, {}, {}, {}

    def alloc_batch(b):
        qT[b] = [
            qkv_pool.tile([128, T], bf16, tag=f"qT{d}", name=f"qT{d}_{b}", bufs=2)
            for d in range(2)
        ]
        kT[b] = [
            qkv_pool.tile([128, T], bf16, tag=f"kT{d}", name=f"kT{d}_{b}", bufs=2)
            for d in range(2)
        ]
        v_t[b] = [
            qkv_pool.tile(
                [128, D_LOC], bf16, tag=f"v{t}", name=f"v{t}_{b}", bufs=2
            )
            for t in range(TK_TILES)
        ]
        avT[b] = [
            av_pool.tile([128, T], bf16, tag=f"avT{d}", name=f"avT{d}_{b}", bufs=2)
            for d in range(2)
        ]

    xt_chunks = {}

    def emit_xt_dma(b, ch):
        t0 = b * T + ch * TQ
        xt = []
        for i in range(C_TILES):
            xti = x_pool.tile([128, TQ], bf16, tag="xt", name=f"xt{b}_{ch}_{i}")
            nc.sync.dma_start(
                out=xti, in_=xT[i * 128 : (i + 1) * 128, t0 : t0 + TQ]
            )
            xt.append(xti)
        return xt

    def emit_qk_unit(b, ch, j, xt):
        # j -> (k0, k1, q0, q1): k first so attention's score deps clear
        # earliest. wqkv column order is q0 q1 k0 k1 v0 v1.
        dest = (kT[b][0], kT[b][1], qT[b][0], qT[b][1])[j]
        wcol = (2, 3, 0, 1)[j]
        ps = ps_po.tile([128, TQ], f32, tag="po", name=f"psqk{b}_{ch}_{j}")
        for i in range(C_TILES):
            nc.tensor.matmul(
                ps,
                w_tiles[i][:, wcol * 128 : (wcol + 1) * 128],
                xt[i],
                start=(i == 0),
                stop=(i == C_TILES - 1),
            )
        # PSUM -> SBUF with per-partition bias add
        nc.vector.tensor_scalar_add(
            dest[:, ch * TQ : (ch + 1) * TQ], ps, bqk_sb[:, wcol : wcol + 1]
        )

    def emit_v_unit(b, ch, ts, xt):
        t_idx = ch * (TQ // 128) + ts
        ps = ps_po.tile([128, TQ], f32, tag="po", name=f"psv{b}_{t_idx}")
        psv = ps[:, :D_LOC]
        for i in range(C_TILES):
            nc.tensor.matmul(
                psv,
                xt[i][:, ts * 128 : (ts + 1) * 128],
                w_tiles[i][:, 2 * D_LOC : 3 * D_LOC],
                start=(i == 0),
                stop=(i == C_TILES - 1),
            )
        nc.vector.tensor_copy(v_t[b][t_idx], psv)

    def emit_attn_unit(b, h, ch):
        # one unit: scores+softmax+AV for one head and one 512-query chunk,
        # k-tiles processed in pairs sharing a [128,1024] score/exp tile
        pav = ps_av.tile([128, TQ], f32, tag="av", name=f"pav{b}_{h}_{ch}")
        acc = acc_pool.tile([128, TQ], bf16, tag="acc", name=f"acc{b}_{h}_{ch}")
        q_sl = qT[b][h][:, ch * TQ : (ch + 1) * TQ]
        for tp in range(TK_TILES // 2):
            pss = ps_s.tile(
                [128, 2 * TQ], f32, tag="s", name=f"pss{b}_{h}_{ch}_{tp}"
            )
            es = es_pool.tile(
                [128, 2 * TQ], bf16, tag="es", name=f"es{b}_{h}_{ch}_{tp}"
            )
            for half in range(2):
                tk = 2 * tp + half
                nc.tensor.matmul(
                    pss[:, half * TQ : (half + 1) * TQ],
                    kT[b][h][:, tk * 128 : (tk + 1) * 128],
                    q_sl,
                    start=True,
                    stop=True,
                )
            nc.scalar.activation(es, pss, Exp, scale=inv_sqrt_dk)
            for half in range(2):
                tk = 2 * tp + half
                nc.tensor.matmul(
                    pav,
                    v_t[b][tk][:, h * 128 : (h + 1) * 128],
                    es[:, half * TQ : (half + 1) * TQ],
                    start=(tk == 0),
                    stop=(tk == TK_TILES - 1),
                )
            with nc.allow_low_precision("softmax denominator partials, bf16"):
                if tp == 0:
                    nc.vector.tensor_copy(acc, es[:, :TQ])
                else:
                    nc.vector.tensor_add(acc, acc, es[:, :TQ])
                nc.vector.tensor_add(acc, acc, es[:, TQ:])
        # single partition-dim reduction of the accumulated exp sums
        pdn = ps_po.tile([128, TQ], f32, tag="po", name=f"pdn{b}_{h}_{ch}")
        nc.tensor.matmul(pdn, ones[:, 0:128], acc, start=True, stop=True)
        rc = rc_pool.tile([128, TQ], f32, tag="rc", name=f"rc{b}_{h}_{ch}")
        nc.vector.reciprocal_approx_fast(out=rc, in_=pdn)
        nc.vector.tensor_mul(avT[b][h][:, ch * TQ : (ch + 1) * TQ], pav, rc)

    def emit_o_t(b, t):
        # output projection for one 128-row tile of y
        row0 = b * T + t * 128
        for half in range(2):
            ystage = y_pool.tile(
                [128, D_MODEL // 2], bf16, tag="ystage", name=f"ys{b}_{t}_{half}"
            )
            for q in range(2):
                nch_i = half * 2 + q
                ps = ps_po.tile(
                    [128, TQ], f32, tag="po", name=f"pso{b}_{t}_{nch_i}"
                )
                for d in range(2):
                    nc.tensor.matmul(
                        ps,
                        avT[b][d][:, t * 128 : (t + 1) * 128],
                        wo_tiles[d][:, nch_i * TQ : (nch_i + 1) * TQ],
                        start=(d == 0),
                        stop=(d == 1),
                    )
                nc.vector.tensor_copy(ystage[:, q * TQ : (q + 1) * TQ], ps)
            nc.sync.dma_start(
                out=y[
                    row0 : row0 + 128,
                    half * (D_MODEL // 2) : (half + 1) * (D_MODEL // 2),
                ],
                in_=ystage,
            )

    # ---------------- S1: projections for batch 0 ----------------
    alloc_batch(0)
    xt_chunks[(0, 0)] = xt_pre
    for ch in range(NCH):
        if ch + 1 < NCH:
            xt_chunks[(0, ch + 1)] = emit_xt_dma(0, ch + 1)  # prefetch
        for j in range(4):
            emit_qk_unit(0, ch, j, xt_chunks[(0, ch)])
        for ts in range(4):
            emit_v_unit(0, ch, ts, xt_chunks[(0, ch)])

    # ---------------- S2: attn(b0) with proj(b1) unit fills -------------
    alloc_batch(1)
    fills = []
    for ch in range(NCH):
        fills.append(("dma", 1, ch))
        for j in range(4):
            fills.append(("qk", 1, ch, j))
        for ts in range(4):
            fills.append(("v", 1, ch, ts))

    def run_fill(f):
        if f[0] == "dma":
            xt_chunks[(f[1], f[2])] = emit_xt_dma(f[1], f[2])
        elif f[0] == "qk":
            emit_qk_unit(f[1], f[2], f[3], xt_chunks[(f[1], f[2])])
        else:
            emit_v_unit(f[1], f[2], f[3], xt_chunks[(f[1], f[2])])

    fi = 0
    for ch in range(NCH):
        for h in range(H_LOC):
            emit_attn_unit(0, h, ch)
            if fi < len(fills):
                run_fill(fills[fi])
                fi += 1
                if fills[fi - 1][0] == "dma" and fi < len(fills):
                    run_fill(fills[fi])
                    fi += 1

    # ---------------- S3: drain proj(b1), then outproj(b0) t 0-7 --------
    while fi < len(fills):
        run_fill(fills[fi])
        fi += 1
    for t in range(8):
        emit_o_t(0, t)

    # ---------------- S4: attn(b1) with outproj fills ----------------
    o_ready = [(0, t) for t in range(8, TK_TILES)]
    oi = 0
    for ch in range(NCH):
        for h in range(H_LOC):
            emit_attn_unit(1, h, ch)
            if h == H_LOC - 1:
                o_ready += [(1, t) for t in range(ch * 4, ch * 4 + 4)]
            take = min(3, len(o_ready) - oi)
            for _ in range(take):
                emit_o_t(*o_ready[oi])
                oi += 1
    while oi < len(o_ready):
        emit_o_t(*o_ready[oi])
        oi += 1


@functools.cache
def _build():
    from concourse import bacc
    import concourse.tile as tile
    from concourse import mybir

    nc = bacc.Bacc(
        "TRN2",
        target_bir_lowering=False,
        debug=False,
        enable_asserts=False,
        num_devices=N_CORES,
    )
    f32 = mybir.dt.float32
    bf16 = mybir.dt.bfloat16
    xT = nc.dram_tensor("xT", [D_MODEL, BT], bf16, kind="ExternalInput").ap()
    wqkv = nc.dram_tensor(
        "wqkv", [D_MODEL, 3 * D_LOC], bf16, kind="ExternalInput"
    ).ap()
    bqk = nc.dram_tensor("bqk", [128, 4], f32, kind="ExternalInput").ap()
    wo = nc.dram_tensor("wo", [D_LOC, D_MODEL], bf16, kind="ExternalInput").ap()
    y = nc.dram_tensor("y", [BT, D_MODEL], bf16, kind="ExternalOutput").ap()

    with tile.TileContext(nc) as tc:
        with ExitStack() as ctx:
            _body(ctx, tc, xT, wqkv, bqk, wo, y)
    nc.compile()
    return nc


def _shard_inputs(x, Wq, bq, Wk, bk, Wv, bv, Wo, bo):
    """Host-side sharding: returns per-core input maps."""
    import ml_dtypes

    bf = ml_dtypes.bfloat16
    f = np.float32
    xT = np.ascontiguousarray(
        np.asarray(x, f).reshape(BT, D_MODEL).T.astype(bf)
    )
    Wq, Wk, Wv, Wo = (np.asarray(a, f) for a in (Wq, Wk, Wv, Wo))
    bq, bk, bv = (np.asarray(a, f) for a in (bq, bk, bv))
    in_maps = []
    for c in range(N_CORES):
        sl = slice(c * D_LOC, (c + 1) * D_LOC)
        wqkv_pad = np.ascontiguousarray(
            np.concatenate([Wq[:, sl], Wk[:, sl], Wv[:, sl]], axis=1).astype(bf)
        )
        bqk_t = np.ascontiguousarray(
            np.stack(
                [
                    bq[sl][:128],
                    bq[sl][128:],
                    bk[sl][:128],
                    bk[sl][128:],
                ],
                axis=1,
            )
        )
        wo_loc = np.ascontiguousarray(Wo[sl, :].astype(bf))
        in_maps.append({"xT": xT, "wqkv": wqkv_pad, "bqk": bqk_t, "wo": wo_loc})
    return in_maps


def _run(in_maps, trace=False, **kwargs):
    from concourse.bass_utils import run_bass_kernel_spmd

    nc = _build()
    return run_bass_kernel_spmd(
        nc, in_maps, core_ids=list(range(N_CORES)), trace=trace, **kwargs
    )


def kernel(x, Wq, bq, Wk, bk, Wv, bv, Wo, bo):
    in_maps = _shard_inputs(x, Wq, bq, Wk, bk, Wv, bv, Wo, bo)
    res = _run(in_maps, trace=False)
    acc = np.zeros((BT, D_MODEL), np.float32)
    for rmap in res.results:
        acc += np.asarray(rmap["y"], dtype=np.float32)
    acc += np.asarray(bo, np.float32)[None, :]
    acc += (np.asarray(bv, np.float32) @ np.asarray(Wo, np.float32))[None, :]
    return acc.reshape(B, T, D_MODEL)
